# revision 25
# baseline (speedup 1.0000x reference)
"""Trainium2 Bass kernel for single-token MoE routing (nn_MixtureOfExperts_v2).

Problem:
    x [2304]; enc_top [256, 2304]; W_down [256, 64, 2304]; encoder_weights
    [256, 512, 64].
    codes = relu_offset(enc_top @ x)           (slope 0.0, offset 1/48)
    top4 values/indices of codes
    per selected expert i (gate v):
        s = W_down[i] @ x                      [64]
        c = relu_offset(E[i] @ s, slope 0.01)  [512]
        d = E[i]^T @ c                         [64]
        recon += W_down[i]^T @ d               [2304]
        recon += v * enc_top[i]
    output = recon                             [2304]

Distribution (8 cores, no collectives): identical SPMD program; per-core
constant tables select the core's role.  Core c handles top-k slot (c % 4)
and output half (c // 4); the host sums the 8 partial outputs.

v9 design (changes from v3 marked *):
  - Routing phase in fp8-e4m3 (enc_top + x): halves routing HBM bytes.
    The gate value is recomputed in bf16 (not from the fp8 PSUM).
  - * Routing matmuls use the DoubleRow fp8 perf mode: 9 chunk-pair
    matmuls at 0.5 cycles/row instead of 18 at 1.0.  Enc tables are
    regrouped (2, 6, 6, 4) so every table holds an even chunk count;
    the x-pair block keeps the k-pair 16 columns apart (dual-fp8
    LDWEIGHTS stride rule).
  - Expert weights gathered in bf16; all expert matmuls bf16 with fp32
    PSUM accumulation (rel-err budget 2e-2; measured ~3.8e-3).
  - * s = W@x as 18 direct PE matmuls (lhsT = W natural slab [128, 64],
    rhs = x chunk column): drops the v3 DVE broadcast-multiply, its
    295KB host-expanded x table, and the DVE->PE dependency.
  - Leaky-offset relu approximated by max(c, 0.01c) (2 DVE ops; exact for
    c outside [0, offset); rel-err unchanged).
  - * Gather order: gW1 (sync) and gW2 (scalar) issue first on the two
    HWDGE rings so the s-stage starts as early as possible; gB and gR
    queue behind gW1 on sync; gC on gpsimd (SWDGE row-concat).
  - * PE warm-keeper matmuls sized to bridge the topk+gather gap: HAM
    re-throttles the PE to half clock after idle gaps > ~3.4us, so the
    keeper holds the power state for the expert phase.
  - * Phase-A enc load spread across all three DMA queues (~200KB each):
    sync enc_a+e_c2, scalar e_c1, gpsimd e_b (SWDGE row-concat gives it
    4.6KB descriptors).  Keep DMA rows >= ~1.5KB: each [128, N] DMA
    costs 128 descriptors regardless of N, so column-splitting tables
    always loses.
  - Output DMA issued raw after the tile-context barrier with no
    completion wait: the multi-microsecond NEFF semaphore-restore
    epilogue covers the 4.6KB transfer's flight time.
"""

import os

import numpy as np
import ml_dtypes

import concourse.bacc as bacc
import concourse.bass as bass
import concourse.mybir as mybir
import concourse.tile as tile
from concourse.bass_utils import run_bass_kernel_spmd

# ---- problem constants (hardcoded per harness contract) ----
IN_DIM = 2304
SUB = 64
ATOMS = 512
NE = 256
K = 4
P = 128
NCHUNK = IN_DIM // P          # 18 chunks of 128 along input dim
HALF = NCHUNK // 2            # 9 chunks per core-half
ACHUNK = ATOMS // P           # 4 chunks of 128 along atoms
N_CORES = 8

WCOLS = SUB * HALF            # 576 cols per W half-table
CCOLS = ACHUNK * SUB + NCHUNK  # 274: E-natural block + enc row

# enc chunk split (all even so DoubleRow pairs stay within one table):
# packing order a | b | c1 | c2 ; consumption order a, c1, b, c2
ENC_A = 2
ENC_B = 6
ENC_C1 = 6
ENC_C2 = 4
# x-pair block: col q = x chunk at position 2q, col 16+q = position 2q+1
# (the dual-fp8 LDWEIGHTS requires the k-pair 16 columns apart)
XCOLS = 32

N_PREWARM = int(os.environ.get("KERNEL_PREWARM_MMS", "8"))
N_WARM = int(os.environ.get("KERNEL_WARM_MMS", "18"))
WARM_COLS = int(os.environ.get("KERNEL_WARM_COLS", "256"))
RAW_OUT_DMA = os.environ.get("KERNEL_RAW_OUT", "1") == "1"

OFFSET = float(np.float32(1.0) / np.float32(48.0))  # 1/sqrt(2304), fp32

F32 = mybir.dt.float32
BF16 = mybir.dt.bfloat16
FP8 = mybir.dt.float8e4
U32 = mybir.dt.uint32


def build_program():
    nc = bacc.Bacc("TRN2", target_bir_lowering=False, debug=False,
                   enable_partition_id=False)

    # routing inputs (fp8); enc_a carries x-fp8 in its first NCHUNK cols
    enc_a = nc.dram_tensor("enc_a", [P, XCOLS + ENC_A * NE], FP8,
                           kind="ExternalInput")
    enc_b = nc.dram_tensor("enc_b", [P, ENC_B, NE], FP8,
                           kind="ExternalInput")
    enc_c1 = nc.dram_tensor("enc_c1", [P, ENC_C1, NE], FP8,
                            kind="ExternalInput")
    enc_c2 = nc.dram_tensor("enc_c2", [P, ENC_C2, NE], FP8,
                            kind="ExternalInput")
    xbf = nc.dram_tensor("xbf", [P, NCHUNK], BF16, kind="ExternalInput")
    slotu = nc.dram_tensor("slotu", [1, 1], U32, kind="ExternalInput")
    # expert tables (bf16)
    tabW1 = nc.dram_tensor("tabw1", [NE, P, WCOLS], BF16,
                           kind="ExternalInput")   # jj-major, own half
    tabW2 = nc.dram_tensor("tabw2", [NE, P, WCOLS], BF16,
                           kind="ExternalInput")   # jj-major, other half
    tabB = nc.dram_tensor("tabb", [NE, SUB, ATOMS], BF16,
                          kind="ExternalInput")    # E^T
    tabC = nc.dram_tensor("tabc", [NE, P, CCOLS], BF16,
                          kind="ExternalInput")    # E-natural + enc row
    tabR = nc.dram_tensor("tabr", [NE, SUB + 1, HALF * P], BF16,
                          kind="ExternalInput")    # [W-nat | enc_row], own
    out_d = nc.dram_tensor("out", [P, HALF], F32, kind="ExternalOutput")
    final_t = nc.alloc_sbuf_tensor("final_raw", [P, HALF], F32)

    with tile.TileContext(nc) as tc:
        with (
            tc.tile_pool(name="sb", bufs=1) as sb,
            tc.tile_pool(name="enc", bufs=1) as encp,
            tc.tile_pool(name="ps", bufs=1, space="PSUM") as ps,
        ):
            # ---- phase A DMAs across all three rings (~200KB each):
            # sync: enc_a + e_c2; scalar: e_c1 + slot + x; gpsimd: e_b
            # (SWDGE concatenates the contiguous rows into big descriptors)
            e_a = encp.tile([P, XCOLS + ENC_A * NE], FP8, tag="ea")
            nc.sync.dma_start(e_a[:], enc_a[:])
            e_c2 = encp.tile([P, ENC_C2, NE], FP8, tag="ec2")
            nc.sync.dma_start(e_c2[:], enc_c2[:])
            e_c1 = encp.tile([P, ENC_C1, NE], FP8, tag="ec1")
            nc.scalar.dma_start(e_c1[:], enc_c1[:])
            e_b = encp.tile([P, ENC_B, NE], FP8, tag="eb")
            nc.gpsimd.dma_start(e_b[:], enc_b[:])
            slot_t = sb.tile([1, 1], U32, tag="slot")
            nc.scalar.dma_start(slot_t[:], slotu[:])
            x_t = sb.tile([P, NCHUNK], BF16, tag="x")
            nc.scalar.dma_start(x_t[:], xbf[:])

            # on-device constants (zwarm first: the PE prewarm waits on it)
            zwarm = sb.tile([P, max(P, WARM_COLS)], BF16, tag="zwarm")
            nc.vector.memset(zwarm[:], 0.0)
            ones_c = sb.tile([P, 1], BF16, tag="onesc")
            nc.vector.memset(ones_c[:], 1.0)
            ones_r = sb.tile([1, P], BF16, tag="onesr")
            nc.vector.memset(ones_r[:], 1.0)

            # ---- PE pre-warm while the first enc DMA is in flight ----
            junk_ps = ps.tile([1, NE], F32, tag="junk")
            for w in range(N_PREWARM):
                nc.tensor.matmul(
                    junk_ps[:, 0:WARM_COLS],
                    lhsT=zwarm[:, 0:1],
                    rhs=zwarm[:, 0:WARM_COLS],
                    start=(w == 0),
                    stop=(w == N_PREWARM - 1),
                )

            # ---- phase A: codes = enc_top @ x (fp8 DoubleRow, PE) ----
            # each matmul consumes a chunk PAIR: lhsT [128, 2, 1] from the
            # x-pair block (k-pair 16 cols apart per the dual-fp8 LDWEIGHTS
            # stride rule), rhs [128, 2, 256] enc slabs, 0.5 cycles/row.
            xp = e_a[:, 0:XCOLS]
            codes_ps = ps.tile([1, NE], F32, tag="codes")
            # packing order: a | b | c1 | c2 ; consumption order below
            groups = [
                ("a", e_a, 0, ENC_A),
                ("c1", e_c1, ENC_A + ENC_B, ENC_C1),
                ("b", e_b, ENC_A, ENC_B),
                ("c2", e_c2, ENC_A + ENC_B + ENC_C1, ENC_C2),
            ]
            n_pairs = NCHUNK // 2
            p_done = 0
            for g, t, base, n in groups:
                for po in range(n // 2):
                    q = (base + 2 * po) // 2     # global pair index
                    if g == "a":
                        rhs = t[:, XCOLS + 2 * po * NE:
                                XCOLS + (2 * po + 2) * NE].rearrange(
                            "p (two n) -> p two n", n=NE)
                    else:
                        rhs = t[:, 2 * po:2 * po + 2, :]
                    lhsT = xp[:, q:q + 17:16].rearrange(
                        "p (two one) -> p two one", one=1)
                    nc.tensor.matmul(
                        codes_ps[:],
                        lhsT=lhsT,
                        rhs=rhs,
                        start=(p_done == 0),
                        stop=(p_done == n_pairs - 1),
                        perf_mode=mybir.MatmulPerfMode.DoubleRow,
                    )
                    p_done += 1

            # ---- phase B: top-8 + slot select via dynamic offsets ----
            slotv = nc.values_load(
                slot_t[:],
                engines={mybir.EngineType.SP, mybir.EngineType.Activation,
                         mybir.EngineType.Pool},
                min_val=0, max_val=K - 1, skip_runtime_bounds_check=True,
            )
            vals = sb.tile([1, 8], F32, tag="vals")
            idxs = sb.tile([1, 8], U32, tag="idxs")
            nc.vector.max_with_indices(vals[:], idxs[:], codes_ps[:])
            val = nc.values_load(
                idxs[:, bass.ds(slotv, 1)],
                engines={mybir.EngineType.SP, mybir.EngineType.Activation,
                         mybir.EngineType.Pool},
                min_val=0, max_val=NE - 1, skip_runtime_bounds_check=True,
            )

            # ---- phase C: expert gather, three rings; gW first ----
            # (HWDGE for gW: the single SWDGE queue serializes its gathers
            # and each dynamic gather pays ~2us Q7 gen+first-byte)
            gW1 = sb.tile([P, WCOLS], BF16, tag="gw1")
            nc.sync.dma_start(gW1[:], tabW1[bass.ds(val, 1), :, :])
            gW2 = sb.tile([P, WCOLS], BF16, tag="gw2")
            nc.scalar.dma_start(gW2[:], tabW2[bass.ds(val, 1), :, :])
            gB = sb.tile([SUB, ATOMS], BF16, tag="gb")
            nc.sync.dma_start(gB[:], tabB[bass.ds(val, 1), :, :])
            gC = sb.tile([P, CCOLS], BF16, tag="gc")
            nc.gpsimd.dma_start(gC[:], tabC[bass.ds(val, 1), :, :])
            gR = sb.tile([SUB + 1, HALF * P], BF16, tag="gr")
            nc.sync.dma_start(gR[:], tabR[bass.ds(val, 1), :, :])

            # ---- PE warm-keeper across the topk+gather gap ----
            # wide moving operand so the keeper holds the HAM power state
            # (idle gaps > ~3.4us re-throttle the PE to half clock)
            if N_WARM:
                for w in range(N_WARM):
                    nc.tensor.matmul(
                        junk_ps[:, 0:WARM_COLS],
                        lhsT=zwarm[:, 0:1],
                        rhs=zwarm[:, 0:WARM_COLS],
                        start=(w == 0),
                        stop=(w == N_WARM - 1),
                    )

            # ---- phase D: expert pipeline (bf16, fp32 PSUM) ----
            # s = W @ x: direct PE matmuls, lhsT = W natural slab
            # [128 p, 64 m], rhs = x chunk column [128, 1]; chunk order
            # own half (gW1) then other half (gW2) to chase the gather.
            s_ps = ps.tile([SUB, 1], F32, tag="s")
            for hi, gw in enumerate((gW1, gW2)):
                for j in range(HALF):
                    jj = hi * HALF + j
                    nc.tensor.matmul(
                        s_ps[:],
                        lhsT=gw[:, j * SUB:(j + 1) * SUB],
                        rhs=x_t[:, jj:jj + 1],
                        start=(hi == 0 and j == 0),
                        stop=(hi == 1 and j == HALF - 1),
                    )
            s_sb = sb.tile([SUB, 1], BF16, tag="ssb")
            nc.vector.tensor_copy(s_sb[:], s_ps[:])

            # c = E @ s : 4 chunks of 128 atoms (lhsT = E^T slabs)
            c_ps = ps.tile([P, ACHUNK], F32, tag="c")
            for ck in range(ACHUNK):
                nc.tensor.matmul(
                    c_ps[:, ck:ck + 1],
                    lhsT=gB[:, ck * P:(ck + 1) * P],
                    rhs=s_sb[:],
                    start=True, stop=True,
                )
            # leaky-offset relu ~ max(c, 0.01c): exact except tiny band
            cleak = sb.tile([P, ACHUNK], F32, tag="cleak")
            nc.vector.tensor_scalar(
                out=cleak[:], in0=c_ps[:], scalar1=0.01, scalar2=None,
                op0=mybir.AluOpType.mult,
            )
            crelu = sb.tile([P, ACHUNK], BF16, tag="crelu")
            nc.vector.tensor_tensor(
                out=crelu[:], in0=c_ps[:], in1=cleak[:],
                op=mybir.AluOpType.max,
            )

            # gate v = relu_off(enc_row . x) in bf16 (off critical path)
            vprod = sb.tile([P, NCHUNK], BF16, tag="vprod")
            nc.vector.tensor_tensor(
                out=vprod[:], in0=gC[:, ACHUNK * SUB:ACHUNK * SUB + NCHUNK],
                in1=x_t[:], op=mybir.AluOpType.mult,
            )
            vred = sb.tile([P, 1], BF16, tag="vred")
            with nc.allow_low_precision(
                    reason="bf16 partial sums; rel-err budget 2e-2"):
                nc.vector.tensor_reduce(
                    out=vred[:], in_=vprod[:], axis=mybir.AxisListType.X,
                    op=mybir.AluOpType.add,
                )
            v_ps = ps.tile([1, 1], F32, tag="v")
            nc.tensor.matmul(v_ps[:], lhsT=vred[:], rhs=ones_c[:],
                             start=True, stop=True)
            vmask = sb.tile([1, 1], F32, tag="vmask")
            nc.vector.tensor_scalar(
                out=vmask[:], in0=v_ps[:], scalar1=OFFSET, scalar2=None,
                op0=mybir.AluOpType.is_ge,
            )

            # d as a [65,1] PSUM column: d = sum_ck E_ck @ crelu_ck with
            # lhsT = E-natural slabs; row 64 gets the relu'd gate v.
            dv_ps = ps.tile([SUB + 1, 1], F32, tag="dv")
            for ck in range(ACHUNK):
                nc.tensor.matmul(
                    dv_ps[0:SUB, :],
                    lhsT=gC[:, ck * SUB:(ck + 1) * SUB],
                    rhs=crelu[:, ck:ck + 1],
                    start=(ck == 0),
                    stop=(ck == ACHUNK - 1),
                )
            vg = sb.tile([1, 1], BF16, tag="vg")
            nc.vector.tensor_tensor(
                out=vg[:], in0=v_ps[:], in1=vmask[:],
                op=mybir.AluOpType.mult,
            )
            nc.tensor.matmul(dv_ps[SUB:SUB + 1, :], lhsT=vg[:],
                             rhs=ones_r[:, 0:1], start=True, stop=True)
            dv_col = sb.tile([SUB + 1, 1], BF16, tag="dvcol")
            nc.vector.tensor_copy(dv_col[:], dv_ps[:])

            # recon on the PE: recon[:, jj] = tabR-chunk^T @ [d | v]
            # (the enc_row row of tabR folds in the v * enc_top[i] term)
            recon_ps = ps.tile([P, HALF], F32, tag="recon")
            for jj in range(HALF):
                nc.tensor.matmul(
                    recon_ps[:, jj:jj + 1],
                    lhsT=gR[:, jj * P:(jj + 1) * P],
                    rhs=dv_col[:],
                    start=True, stop=True,
                )
            final_ap = (final_t.ap() if RAW_OUT_DMA
                        else sb.tile([P, HALF], F32, tag="final")[:])
            nc.vector.tensor_copy(final_ap, recon_ps[:])
            if not RAW_OUT_DMA:
                nc.sync.dma_start(out_d[:], final_ap)

    if RAW_OUT_DMA:
        # raw out DMA after the tile-context barrier with no completion
        # wait: the multi-microsecond NEFF semaphore-restore epilogue
        # covers the 4.6KB transfer's flight time.  (Issuing it inside
        # the context loses ~1.5us: tile assigns the DMA a DMAHW lane
        # and the exit barrier waits for its completion.)
        out_sem = nc.alloc_semaphore("raw_out_sem")
        nc.sync.dma_start(out_d[:], final_t.ap()).then_inc(out_sem, 16)

    nc.compile()
    return nc


def _chunk_order(h):
    """Chunk visit order for core-half h: own half first."""
    own = list(range(h * HALF, (h + 1) * HALF))
    other = list(range((1 - h) * HALF, (2 - h) * HALF))
    return own + other


def _host_prep(x, enc_top, W_down, encoder_weights):
    """Build per-core-half input tables (pure layout transforms)."""
    bf = ml_dtypes.bfloat16
    f8 = ml_dtypes.float8_e4m3fn
    x = np.asarray(x, np.float32)
    enc_top = np.asarray(enc_top, np.float32)
    W_down = np.asarray(W_down, np.float32)
    E = np.asarray(encoder_weights, np.float32)

    # E^T table [g, m, a]
    tabB = np.ascontiguousarray(E.transpose(0, 2, 1)).astype(bf)
    # E-natural block: [g, p, ck*64+m] = E[g, ck*128+p, m]
    encnat = np.ascontiguousarray(
        E.reshape(NE, ACHUNK, P, SUB).transpose(0, 2, 1, 3)
    ).reshape(NE, P, ACHUNK * SUB)

    Wr = W_down.reshape(NE, SUB, NCHUNK, P)          # [g, m, j, p]
    Er = enc_top.reshape(NE, NCHUNK, P)              # [g, j, p]

    per_half = {}
    for h in (0, 1):
        order = _chunk_order(h)
        o1, o2 = order[:HALF], order[HALF:]
        tabW1 = np.ascontiguousarray(
            Wr[:, :, o1, :].transpose(0, 3, 2, 1)    # [g, p, j, m]
        ).reshape(NE, P, WCOLS).astype(bf)
        tabW2 = np.ascontiguousarray(
            Wr[:, :, o2, :].transpose(0, 3, 2, 1)
        ).reshape(NE, P, WCOLS).astype(bf)
        encrow = Er[:, order, :].transpose(0, 2, 1)  # [g, p, jj] full 18
        tabC = np.concatenate([encnat, encrow], axis=2).astype(bf)
        # [W-natural | enc_row] fused recon table: [g, 65, (j p)]
        wnat = Wr[:, :, o1, :].reshape(NE, SUB, HALF * P)      # [g, m, j*p]
        erow = Er[:, o1, :].reshape(NE, 1, HALF * P)           # [g, 1, j*p]
        tabR = np.ascontiguousarray(
            np.concatenate([wnat, erow], axis=1)).astype(bf)

        x_pm = np.ascontiguousarray(
            x.reshape(NCHUNK, P)[order, :].T)        # [p, jj] fp32
        encf8 = np.ascontiguousarray(
            Er[:, order, :].transpose(2, 1, 0)       # [p, jj, g]
        ).astype(f8)
        xpair = np.zeros((P, XCOLS), np.float32)
        xpair[:, 0:HALF] = x_pm[:, 0::2]       # first chunk of each pair
        xpair[:, 16:16 + HALF] = x_pm[:, 1::2]  # second chunk of each pair
        enc_a = np.concatenate(
            [xpair.astype(f8),
             encf8[:, 0:ENC_A, :].reshape(P, ENC_A * NE)], axis=1)
        n_ab = ENC_A + ENC_B
        n_abc = n_ab + ENC_C1
        per_half[h] = dict(
            tabw1=tabW1, tabw2=tabW2, tabc=tabC, tabr=tabR,
            xbf=x_pm.astype(bf),
            enc_a=np.ascontiguousarray(enc_a),
            enc_b=np.ascontiguousarray(encf8[:, ENC_A:n_ab, :]),
            enc_c1=np.ascontiguousarray(encf8[:, n_ab:n_abc, :]),
            enc_c2=np.ascontiguousarray(encf8[:, n_abc:, :]),
        )

    in_maps = []
    for c in range(N_CORES):
        h, slot = c // 4, c % 4
        ph = per_half[h]
        in_maps.append({
            "enc_a": ph["enc_a"],
            "enc_b": ph["enc_b"],
            "enc_c1": ph["enc_c1"],
            "enc_c2": ph["enc_c2"],
            "xbf": ph["xbf"],
            "slotu": np.array([[slot]], np.uint32),
            "tabw1": ph["tabw1"],
            "tabw2": ph["tabw2"],
            "tabb": tabB,
            "tabc": ph["tabc"],
            "tabr": ph["tabr"],
        })
    return in_maps


def _assemble(results):
    out = np.zeros(IN_DIM, np.float32).reshape(NCHUNK, P)
    for c in range(N_CORES):
        h = c // 4
        own = _chunk_order(h)[:HALF]
        out[own, :] += results[c]["out"].T
    return out.reshape(IN_DIM)


_NC_CACHE = {}
LAST_RESULT = {}


def kernel(x, enc_top, W_down, encoder_weights):
    in_maps = _host_prep(x, enc_top, W_down, encoder_weights)
    if "nc" not in _NC_CACHE:
        _NC_CACHE["nc"] = build_program()
    nc = _NC_CACHE["nc"]

    if os.environ.get("BASS_SIM") == "1":
        from concourse.bass_interp import CoreSim
        sim_cores = os.environ.get("BASS_SIM_CORES")
        cores = (
            [int(t) for t in sim_cores.split(",")] if sim_cores
            else range(N_CORES)
        )
        results = [None] * N_CORES
        for c in cores:
            nc_c = build_program()
            sim = CoreSim(nc_c)
            for name, arr in in_maps[c].items():
                sim.tensor(name)[:] = arr
            sim.simulate()
            results[c] = {"out": np.array(sim.tensor("out"))}
        for c in range(N_CORES):
            if results[c] is None:
                results[c] = {"out": np.zeros((P, HALF), np.float32)}
        return _assemble(results)

    trace = os.environ.get("BASS_TRACE") == "1"
    if trace:
        _ensure_trace_hook()
    res = run_bass_kernel_spmd(
        nc, in_maps, core_ids=list(range(N_CORES)),
        trace=trace,
    )
    LAST_RESULT["res"] = res
    return _assemble(res.results)


def _ensure_trace_hook():
    """Install the axon NTFF profile hook if antenv.axon_hooks is absent."""
    try:
        from antenv.axon_hooks import get_axon_ntff_profile_hook  # noqa
        return
    except ImportError:
        pass
    import sys
    import types
    try:
        from trn_agent_boot.trn_boot import _ntff_profile_via_ctypes
    except ImportError:
        return
    hook = _ntff_profile_via_ctypes("/opt/axon/libaxon_pjrt.so")
    mod = types.ModuleType("antenv.axon_hooks")
    mod._hook = hook
    mod.get_axon_ntff_profile_hook = lambda: mod._hook
    mod.set_axon_ntff_profile_hook = lambda h: setattr(mod, "_hook", h)
    import antenv
    sys.modules["antenv.axon_hooks"] = mod
    antenv.axon_hooks = mod


if __name__ == "__main__":
    nc = build_program()
    print("program built ok")


# revision 26
# speedup vs baseline: 1.1335x; 1.1335x over previous
"""Trainium2 Bass kernel for single-token MoE routing (nn_MixtureOfExperts_v2).

Problem:
    x [2304]; enc_top [256, 2304]; W_down [256, 64, 2304]; encoder_weights
    [256, 512, 64].
    codes = relu_offset(enc_top @ x)           (slope 0.0, offset 1/48)
    top4 values/indices of codes
    per selected expert i (gate v):
        s = W_down[i] @ x                      [64]
        c = relu_offset(E[i] @ s, slope 0.01)  [512]
        d = E[i]^T @ c                         [64]
        recon += W_down[i]^T @ d               [2304]
        recon += v * enc_top[i]
    output = recon                             [2304]

Distribution (8 cores, no collectives): identical SPMD program; per-core
constant tables select the core's role.  Core c handles top-k slot (c % 4)
and output half (c // 4); the host sums the 8 partial outputs.

v9 design (changes from v3 marked *):
  - Routing phase in fp8-e4m3 (enc_top + x): halves routing HBM bytes.
    The gate value is recomputed in bf16 (not from the fp8 PSUM).
  - * Routing matmuls use the DoubleRow fp8 perf mode: 9 chunk-pair
    matmuls at 0.5 cycles/row instead of 18 at 1.0.  Enc tables are
    regrouped (2, 6, 6, 4) so every table holds an even chunk count;
    the x-pair block keeps the k-pair 16 columns apart (dual-fp8
    LDWEIGHTS stride rule).
  - Expert weights gathered in bf16; all expert matmuls bf16 with fp32
    PSUM accumulation (rel-err budget 2e-2; measured ~3.8e-3).
  - * s = W@x as 18 direct PE matmuls (lhsT = W natural slab [128, 64],
    rhs = x chunk column): drops the v3 DVE broadcast-multiply, its
    295KB host-expanded x table, and the DVE->PE dependency.
  - Leaky-offset relu approximated by max(c, 0.01c) (2 DVE ops; exact for
    c outside [0, offset); rel-err unchanged).
  - * Gather order: gW1 (sync) and gW2 (scalar) issue first on the two
    HWDGE rings so the s-stage starts as early as possible; gB and gR
    queue behind gW1 on sync; gC on gpsimd (SWDGE row-concat).
  - * PE warm-keeper matmuls sized to bridge the topk+gather gap: HAM
    re-throttles the PE to half clock after idle gaps > ~3.4us, so the
    keeper holds the power state for the expert phase.
  - * Phase-A enc load spread across all three DMA queues (~200KB each):
    sync enc_a+e_c2, scalar e_c1, gpsimd e_b (SWDGE row-concat gives it
    4.6KB descriptors).  Keep DMA rows >= ~1.5KB: each [128, N] DMA
    costs 128 descriptors regardless of N, so column-splitting tables
    always loses.
  - Output DMA issued raw after the tile-context barrier with no
    completion wait: the multi-microsecond NEFF semaphore-restore
    epilogue covers the 4.6KB transfer's flight time.
"""

import os

import numpy as np
import ml_dtypes

import concourse.bacc as bacc
import concourse.bass as bass
import concourse.mybir as mybir
import concourse.tile as tile
from concourse.bass_utils import run_bass_kernel_spmd

# ---- problem constants (hardcoded per harness contract) ----
IN_DIM = 2304
SUB = 64
ATOMS = 512
NE = 256
K = 4
P = 128
NCHUNK = IN_DIM // P          # 18 chunks of 128 along input dim
HALF = NCHUNK // 2            # 9 chunks per core-half
ACHUNK = ATOMS // P           # 4 chunks of 128 along atoms
N_CORES = 8

WCOLS = SUB * HALF            # 576 cols per W half-table
CCOLS = ACHUNK * SUB + NCHUNK  # 274: E-natural block + enc row

# enc chunk split (all even so DoubleRow pairs stay within one table):
# packing order a | b | c1 | c2 ; consumption order a, c1, b, c2
ENC_A = 2
ENC_B = 6
ENC_C1 = 6
ENC_C2 = 4
# x-pair block: col q = x chunk at position 2q, col 16+q = position 2q+1
# (the dual-fp8 LDWEIGHTS requires the k-pair 16 columns apart)
XCOLS = 32

N_PREWARM = int(os.environ.get("KERNEL_PREWARM_MMS", "8"))
N_WARM = int(os.environ.get("KERNEL_WARM_MMS", "18"))
WARM_COLS = int(os.environ.get("KERNEL_WARM_COLS", "256"))
RAW_OUT_DMA = os.environ.get("KERNEL_RAW_OUT", "1") == "1"

OFFSET = float(np.float32(1.0) / np.float32(48.0))  # 1/sqrt(2304), fp32

F32 = mybir.dt.float32
BF16 = mybir.dt.bfloat16
FP8 = mybir.dt.float8e4
U32 = mybir.dt.uint32


def build_program():
    nc = bacc.Bacc("TRN2", target_bir_lowering=False, debug=False,
                   enable_partition_id=False)

    # routing inputs (fp8); enc_a carries x-fp8 in its first NCHUNK cols
    enc_a = nc.dram_tensor("enc_a", [P, XCOLS + ENC_A * NE], FP8,
                           kind="ExternalInput")
    enc_b = nc.dram_tensor("enc_b", [P, ENC_B, NE], FP8,
                           kind="ExternalInput")
    enc_c1 = nc.dram_tensor("enc_c1", [P, ENC_C1, NE], FP8,
                            kind="ExternalInput")
    enc_c2 = nc.dram_tensor("enc_c2", [P, ENC_C2, NE], FP8,
                            kind="ExternalInput")
    xbf = nc.dram_tensor("xbf", [P, NCHUNK], BF16, kind="ExternalInput")
    slotu = nc.dram_tensor("slotu", [1, 1], U32, kind="ExternalInput")
    # expert tables (bf16)
    tabW1 = nc.dram_tensor("tabw1", [NE, P, WCOLS], BF16,
                           kind="ExternalInput")   # jj-major, own half
    tabW2 = nc.dram_tensor("tabw2", [NE, P, WCOLS], BF16,
                           kind="ExternalInput")   # jj-major, other half
    tabB = nc.dram_tensor("tabb", [NE, SUB, ATOMS], BF16,
                          kind="ExternalInput")    # E^T
    tabC = nc.dram_tensor("tabc", [NE, P, CCOLS], BF16,
                          kind="ExternalInput")    # E-natural + enc row
    tabR = nc.dram_tensor("tabr", [NE, SUB + 1, HALF * P], BF16,
                          kind="ExternalInput")    # [W-nat | enc_row], own
    out_d = nc.dram_tensor("out", [P, HALF], F32, kind="ExternalOutput")
    final_t = nc.alloc_sbuf_tensor("final_raw", [P, HALF], F32)

    with tile.TileContext(nc) as tc:
        with (
            tc.tile_pool(name="sb", bufs=1) as sb,
            tc.tile_pool(name="enc", bufs=1) as encp,
            tc.tile_pool(name="ps", bufs=1, space="PSUM") as ps,
        ):
            # ---- phase A DMAs across all three rings:
            # sync: enc_a + e_b; scalar: e_c1 + slot + x; gpsimd: e_c2
            # (SWDGE concatenates the contiguous rows into big
            # descriptors; the smallest group rides the slowest queue)
            e_a = encp.tile([P, XCOLS + ENC_A * NE], FP8, tag="ea")
            nc.sync.dma_start(e_a[:], enc_a[:])
            e_b = encp.tile([P, ENC_B, NE], FP8, tag="eb")
            nc.sync.dma_start(e_b[:], enc_b[:])
            e_c1 = encp.tile([P, ENC_C1, NE], FP8, tag="ec1")
            nc.scalar.dma_start(e_c1[:], enc_c1[:])
            e_c2 = encp.tile([P, ENC_C2, NE], FP8, tag="ec2")
            nc.gpsimd.dma_start(e_c2[:], enc_c2[:])
            slot_t = sb.tile([1, 1], U32, tag="slot")
            nc.scalar.dma_start(slot_t[:], slotu[:])
            x_t = sb.tile([P, NCHUNK], BF16, tag="x")
            nc.scalar.dma_start(x_t[:], xbf[:])

            # on-device constants (zwarm first: the PE prewarm waits on it)
            zwarm = sb.tile([P, max(P, WARM_COLS)], BF16, tag="zwarm")
            nc.vector.memset(zwarm[:], 0.0)
            ones_c = sb.tile([P, 1], BF16, tag="onesc")
            nc.vector.memset(ones_c[:], 1.0)
            ones_r = sb.tile([1, P], BF16, tag="onesr")
            nc.vector.memset(ones_r[:], 1.0)

            # ---- PE pre-warm while the first enc DMA is in flight ----
            junk_ps = ps.tile([1, NE], F32, tag="junk")
            for w in range(N_PREWARM):
                nc.tensor.matmul(
                    junk_ps[:, 0:WARM_COLS],
                    lhsT=zwarm[:, 0:1],
                    rhs=zwarm[:, 0:WARM_COLS],
                    start=(w == 0),
                    stop=(w == N_PREWARM - 1),
                )

            # ---- phase A: codes = enc_top @ x (fp8 DoubleRow, PE) ----
            # each matmul consumes a chunk PAIR: lhsT [128, 2, 1] from the
            # x-pair block (k-pair 16 cols apart per the dual-fp8 LDWEIGHTS
            # stride rule), rhs [128, 2, 256] enc slabs, 0.5 cycles/row.
            xp = e_a[:, 0:XCOLS]
            codes_ps = ps.tile([1, NE], F32, tag="codes")
            # packing order: a | b | c1 | c2 ; consumption order below
            # follows expected DMA arrival (a, c1, c2, then sync's b)
            groups = [
                ("a", e_a, 0, ENC_A),
                ("c1", e_c1, ENC_A + ENC_B, ENC_C1),
                ("c2", e_c2, ENC_A + ENC_B + ENC_C1, ENC_C2),
                ("b", e_b, ENC_A, ENC_B),
            ]
            n_pairs = NCHUNK // 2
            p_done = 0
            for g, t, base, n in groups:
                for po in range(n // 2):
                    q = (base + 2 * po) // 2     # global pair index
                    if g == "a":
                        rhs = t[:, XCOLS + 2 * po * NE:
                                XCOLS + (2 * po + 2) * NE].rearrange(
                            "p (two n) -> p two n", n=NE)
                    else:
                        rhs = t[:, 2 * po:2 * po + 2, :]
                    lhsT = xp[:, q:q + 17:16].rearrange(
                        "p (two one) -> p two one", one=1)
                    nc.tensor.matmul(
                        codes_ps[:],
                        lhsT=lhsT,
                        rhs=rhs,
                        start=(p_done == 0),
                        stop=(p_done == n_pairs - 1),
                        perf_mode=mybir.MatmulPerfMode.DoubleRow,
                    )
                    p_done += 1

            # ---- phase B: top-8 + slot select via dynamic offsets ----
            slotv = nc.values_load(
                slot_t[:],
                engines={mybir.EngineType.SP, mybir.EngineType.Activation,
                         mybir.EngineType.Pool},
                min_val=0, max_val=K - 1, skip_runtime_bounds_check=True,
            )
            vals = sb.tile([1, 8], F32, tag="vals")
            idxs = sb.tile([1, 8], U32, tag="idxs")
            nc.vector.max_with_indices(vals[:], idxs[:], codes_ps[:])
            val = nc.values_load(
                idxs[:, bass.ds(slotv, 1)],
                engines={mybir.EngineType.SP, mybir.EngineType.Activation,
                         mybir.EngineType.Pool},
                min_val=0, max_val=NE - 1, skip_runtime_bounds_check=True,
            )

            # ---- phase C: expert gather, three rings; gW first ----
            # (HWDGE for gW: the single SWDGE queue serializes its gathers
            # and each dynamic gather pays ~2us Q7 gen+first-byte)
            gW1 = sb.tile([P, WCOLS], BF16, tag="gw1")
            nc.sync.dma_start(gW1[:], tabW1[bass.ds(val, 1), :, :])
            gW2 = sb.tile([P, WCOLS], BF16, tag="gw2")
            nc.scalar.dma_start(gW2[:], tabW2[bass.ds(val, 1), :, :])
            gB = sb.tile([SUB, ATOMS], BF16, tag="gb")
            nc.sync.dma_start(gB[:], tabB[bass.ds(val, 1), :, :])
            gC = sb.tile([P, CCOLS], BF16, tag="gc")
            nc.gpsimd.dma_start(gC[:], tabC[bass.ds(val, 1), :, :])
            gR = sb.tile([SUB + 1, HALF * P], BF16, tag="gr")
            nc.sync.dma_start(gR[:], tabR[bass.ds(val, 1), :, :])

            # ---- PE warm-keeper across the topk+gather gap ----
            # wide moving operand so the keeper holds the HAM power state
            # (idle gaps > ~3.4us re-throttle the PE to half clock)
            if N_WARM:
                for w in range(N_WARM):
                    nc.tensor.matmul(
                        junk_ps[:, 0:WARM_COLS],
                        lhsT=zwarm[:, 0:1],
                        rhs=zwarm[:, 0:WARM_COLS],
                        start=(w == 0),
                        stop=(w == N_WARM - 1),
                    )

            # ---- phase D: expert pipeline (bf16, fp32 PSUM) ----
            # s = W @ x: direct PE matmuls, lhsT = W natural slab
            # [128 p, 64 m], rhs = x chunk column [128, 1]; chunk order
            # own half (gW1) then other half (gW2) to chase the gather.
            s_ps = ps.tile([SUB, 1], F32, tag="s")
            for hi, gw in enumerate((gW1, gW2)):
                for j in range(HALF):
                    jj = hi * HALF + j
                    nc.tensor.matmul(
                        s_ps[:],
                        lhsT=gw[:, j * SUB:(j + 1) * SUB],
                        rhs=x_t[:, jj:jj + 1],
                        start=(hi == 0 and j == 0),
                        stop=(hi == 1 and j == HALF - 1),
                    )
            s_sb = sb.tile([SUB, 1], BF16, tag="ssb")
            nc.vector.tensor_copy(s_sb[:], s_ps[:])

            # c = E @ s : 4 chunks of 128 atoms (lhsT = E^T slabs)
            c_ps = ps.tile([P, ACHUNK], F32, tag="c")
            for ck in range(ACHUNK):
                nc.tensor.matmul(
                    c_ps[:, ck:ck + 1],
                    lhsT=gB[:, ck * P:(ck + 1) * P],
                    rhs=s_sb[:],
                    start=True, stop=True,
                )
            # leaky-offset relu ~ max(c, 0.01c): exact except tiny band
            cleak = sb.tile([P, ACHUNK], F32, tag="cleak")
            nc.vector.tensor_scalar(
                out=cleak[:], in0=c_ps[:], scalar1=0.01, scalar2=None,
                op0=mybir.AluOpType.mult,
            )
            crelu = sb.tile([P, ACHUNK], BF16, tag="crelu")
            nc.vector.tensor_tensor(
                out=crelu[:], in0=c_ps[:], in1=cleak[:],
                op=mybir.AluOpType.max,
            )

            # gate v = relu_off(enc_row . x) in bf16 (off critical path)
            vprod = sb.tile([P, NCHUNK], BF16, tag="vprod")
            nc.vector.tensor_tensor(
                out=vprod[:], in0=gC[:, ACHUNK * SUB:ACHUNK * SUB + NCHUNK],
                in1=x_t[:], op=mybir.AluOpType.mult,
            )
            vred = sb.tile([P, 1], BF16, tag="vred")
            with nc.allow_low_precision(
                    reason="bf16 partial sums; rel-err budget 2e-2"):
                nc.vector.tensor_reduce(
                    out=vred[:], in_=vprod[:], axis=mybir.AxisListType.X,
                    op=mybir.AluOpType.add,
                )
            v_ps = ps.tile([1, 1], F32, tag="v")
            nc.tensor.matmul(v_ps[:], lhsT=vred[:], rhs=ones_c[:],
                             start=True, stop=True)
            vmask = sb.tile([1, 1], F32, tag="vmask")
            nc.vector.tensor_scalar(
                out=vmask[:], in0=v_ps[:], scalar1=OFFSET, scalar2=None,
                op0=mybir.AluOpType.is_ge,
            )

            # d as a [65,1] PSUM column: d = sum_ck E_ck @ crelu_ck with
            # lhsT = E-natural slabs; row 64 gets the relu'd gate v.
            dv_ps = ps.tile([SUB + 1, 1], F32, tag="dv")
            for ck in range(ACHUNK):
                nc.tensor.matmul(
                    dv_ps[0:SUB, :],
                    lhsT=gC[:, ck * SUB:(ck + 1) * SUB],
                    rhs=crelu[:, ck:ck + 1],
                    start=(ck == 0),
                    stop=(ck == ACHUNK - 1),
                )
            vg = sb.tile([1, 1], BF16, tag="vg")
            nc.vector.tensor_tensor(
                out=vg[:], in0=v_ps[:], in1=vmask[:],
                op=mybir.AluOpType.mult,
            )
            nc.tensor.matmul(dv_ps[SUB:SUB + 1, :], lhsT=vg[:],
                             rhs=ones_r[:, 0:1], start=True, stop=True)
            dv_col = sb.tile([SUB + 1, 1], BF16, tag="dvcol")
            nc.vector.tensor_copy(dv_col[:], dv_ps[:])

            # recon on the PE: recon[:, jj] = tabR-chunk^T @ [d | v]
            # (the enc_row row of tabR folds in the v * enc_top[i] term)
            recon_ps = ps.tile([P, HALF], F32, tag="recon")
            for jj in range(HALF):
                nc.tensor.matmul(
                    recon_ps[:, jj:jj + 1],
                    lhsT=gR[:, jj * P:(jj + 1) * P],
                    rhs=dv_col[:],
                    start=True, stop=True,
                )
            final_ap = (final_t.ap() if RAW_OUT_DMA
                        else sb.tile([P, HALF], F32, tag="final")[:])
            nc.vector.tensor_copy(final_ap, recon_ps[:])
            if not RAW_OUT_DMA:
                nc.sync.dma_start(out_d[:], final_ap)

    if RAW_OUT_DMA:
        # raw out DMA after the tile-context barrier with no completion
        # wait: the multi-microsecond NEFF semaphore-restore epilogue
        # covers the 4.6KB transfer's flight time.  (Issuing it inside
        # the context loses ~1.5us: tile assigns the DMA a DMAHW lane
        # and the exit barrier waits for its completion.)
        out_sem = nc.alloc_semaphore("raw_out_sem")
        nc.sync.dma_start(out_d[:], final_t.ap()).then_inc(out_sem, 16)

    nc.compile()
    return nc


def _chunk_order(h):
    """Chunk visit order for core-half h: own half first."""
    own = list(range(h * HALF, (h + 1) * HALF))
    other = list(range((1 - h) * HALF, (2 - h) * HALF))
    return own + other


def _host_prep(x, enc_top, W_down, encoder_weights):
    """Build per-core-half input tables (pure layout transforms)."""
    bf = ml_dtypes.bfloat16
    f8 = ml_dtypes.float8_e4m3fn
    x = np.asarray(x, np.float32)
    enc_top = np.asarray(enc_top, np.float32)
    W_down = np.asarray(W_down, np.float32)
    E = np.asarray(encoder_weights, np.float32)

    # E^T table [g, m, a]
    tabB = np.ascontiguousarray(E.transpose(0, 2, 1)).astype(bf)
    # E-natural block: [g, p, ck*64+m] = E[g, ck*128+p, m]
    encnat = np.ascontiguousarray(
        E.reshape(NE, ACHUNK, P, SUB).transpose(0, 2, 1, 3)
    ).reshape(NE, P, ACHUNK * SUB)

    Wr = W_down.reshape(NE, SUB, NCHUNK, P)          # [g, m, j, p]
    Er = enc_top.reshape(NE, NCHUNK, P)              # [g, j, p]

    per_half = {}
    for h in (0, 1):
        order = _chunk_order(h)
        o1, o2 = order[:HALF], order[HALF:]
        tabW1 = np.ascontiguousarray(
            Wr[:, :, o1, :].transpose(0, 3, 2, 1)    # [g, p, j, m]
        ).reshape(NE, P, WCOLS).astype(bf)
        tabW2 = np.ascontiguousarray(
            Wr[:, :, o2, :].transpose(0, 3, 2, 1)
        ).reshape(NE, P, WCOLS).astype(bf)
        encrow = Er[:, order, :].transpose(0, 2, 1)  # [g, p, jj] full 18
        tabC = np.concatenate([encnat, encrow], axis=2).astype(bf)
        # [W-natural | enc_row] fused recon table: [g, 65, (j p)]
        wnat = Wr[:, :, o1, :].reshape(NE, SUB, HALF * P)      # [g, m, j*p]
        erow = Er[:, o1, :].reshape(NE, 1, HALF * P)           # [g, 1, j*p]
        tabR = np.ascontiguousarray(
            np.concatenate([wnat, erow], axis=1)).astype(bf)

        x_pm = np.ascontiguousarray(
            x.reshape(NCHUNK, P)[order, :].T)        # [p, jj] fp32
        encf8 = np.ascontiguousarray(
            Er[:, order, :].transpose(2, 1, 0)       # [p, jj, g]
        ).astype(f8)
        xpair = np.zeros((P, XCOLS), np.float32)
        xpair[:, 0:HALF] = x_pm[:, 0::2]       # first chunk of each pair
        xpair[:, 16:16 + HALF] = x_pm[:, 1::2]  # second chunk of each pair
        enc_a = np.concatenate(
            [xpair.astype(f8),
             encf8[:, 0:ENC_A, :].reshape(P, ENC_A * NE)], axis=1)
        n_ab = ENC_A + ENC_B
        n_abc = n_ab + ENC_C1
        per_half[h] = dict(
            tabw1=tabW1, tabw2=tabW2, tabc=tabC, tabr=tabR,
            xbf=x_pm.astype(bf),
            enc_a=np.ascontiguousarray(enc_a),
            enc_b=np.ascontiguousarray(encf8[:, ENC_A:n_ab, :]),
            enc_c1=np.ascontiguousarray(encf8[:, n_ab:n_abc, :]),
            enc_c2=np.ascontiguousarray(encf8[:, n_abc:, :]),
        )

    in_maps = []
    for c in range(N_CORES):
        h, slot = c // 4, c % 4
        ph = per_half[h]
        in_maps.append({
            "enc_a": ph["enc_a"],
            "enc_b": ph["enc_b"],
            "enc_c1": ph["enc_c1"],
            "enc_c2": ph["enc_c2"],
            "xbf": ph["xbf"],
            "slotu": np.array([[slot]], np.uint32),
            "tabw1": ph["tabw1"],
            "tabw2": ph["tabw2"],
            "tabb": tabB,
            "tabc": ph["tabc"],
            "tabr": ph["tabr"],
        })
    return in_maps


def _assemble(results):
    out = np.zeros(IN_DIM, np.float32).reshape(NCHUNK, P)
    for c in range(N_CORES):
        h = c // 4
        own = _chunk_order(h)[:HALF]
        out[own, :] += results[c]["out"].T
    return out.reshape(IN_DIM)


_NC_CACHE = {}
LAST_RESULT = {}


def kernel(x, enc_top, W_down, encoder_weights):
    in_maps = _host_prep(x, enc_top, W_down, encoder_weights)
    if "nc" not in _NC_CACHE:
        _NC_CACHE["nc"] = build_program()
    nc = _NC_CACHE["nc"]

    if os.environ.get("BASS_SIM") == "1":
        from concourse.bass_interp import CoreSim
        sim_cores = os.environ.get("BASS_SIM_CORES")
        cores = (
            [int(t) for t in sim_cores.split(",")] if sim_cores
            else range(N_CORES)
        )
        results = [None] * N_CORES
        for c in cores:
            nc_c = build_program()
            sim = CoreSim(nc_c)
            for name, arr in in_maps[c].items():
                sim.tensor(name)[:] = arr
            sim.simulate()
            results[c] = {"out": np.array(sim.tensor("out"))}
        for c in range(N_CORES):
            if results[c] is None:
                results[c] = {"out": np.zeros((P, HALF), np.float32)}
        return _assemble(results)

    trace = os.environ.get("BASS_TRACE") == "1"
    if trace:
        _ensure_trace_hook()
    res = run_bass_kernel_spmd(
        nc, in_maps, core_ids=list(range(N_CORES)),
        trace=trace,
    )
    LAST_RESULT["res"] = res
    return _assemble(res.results)


def _ensure_trace_hook():
    """Install the axon NTFF profile hook if antenv.axon_hooks is absent."""
    try:
        from antenv.axon_hooks import get_axon_ntff_profile_hook  # noqa
        return
    except ImportError:
        pass
    import sys
    import types
    try:
        from trn_agent_boot.trn_boot import _ntff_profile_via_ctypes
    except ImportError:
        return
    hook = _ntff_profile_via_ctypes("/opt/axon/libaxon_pjrt.so")
    mod = types.ModuleType("antenv.axon_hooks")
    mod._hook = hook
    mod.get_axon_ntff_profile_hook = lambda: mod._hook
    mod.set_axon_ntff_profile_hook = lambda h: setattr(mod, "_hook", h)
    import antenv
    sys.modules["antenv.axon_hooks"] = mod
    antenv.axon_hooks = mod


if __name__ == "__main__":
    nc = build_program()
    print("program built ok")
